# revision 10
# baseline (speedup 1.0000x reference)
"""GQA attention kernel for Trainium2, tensor-parallel across 8 NeuronCores.

Problem: B=2, T=2048, D=2048, H=32 q-heads, G=8 kv-heads (GQA, rep=4), hd=64,
causal softmax attention + output projection, fp32 I/O.

Sharding (one KV group per core):
  core g: Wq[:, g*256:(g+1)*256], Wk/Wv[:, g*64:(g+1)*64], Wo[g*256:(g+1)*256, :]
  Each core computes its 4 heads' attention + partial output projection;
  host sums the 8 partial outputs (row-parallel Wo => partial-sum unshard).

On-device dataflow per core (all matmuls contract over the partition dim):
  QT = wq.T @ xT        [256, T]  (fp32r, full PE rate; 1/8 scale folded in)
  [KT; VT] = wkv.T @ xT [128, T]
  KT duplicated to partitions 64..127 so two heads' score matmuls run
  concurrently in disjoint PE row groups (contraction = hd = 64).
  ST_r = K @ QT_r       [128k, 512q] blocks (fp32r), causal block-skipped
  PT_r = exp(ST_r)      (ACT, bf16 out; diag blocks masked via affine_select)
  OT_r = [V|1].T @ PT_r [65, 512] (bf16, psum-accumulated over k tiles;
                         row 64 = softmax denominators)
  OT normalized by 1/denominator (DVE), stored as Wo lhsT layout.
  partial = OT.T @ wo   [t, 2048] (fp32r), DMA'd straight from PSUM to DRAM.
"""

import os
import sys

import numpy as np

for _p in ("/opt/trn_rl_repo", "/root/.axon_site/_ro/trn_rl_repo"):
    if os.path.isdir(_p) and _p not in sys.path:
        sys.path.insert(0, _p)

import concourse.bass as bass  # noqa: E402
import concourse.mybir as mybir  # noqa: E402
import concourse.tile as tile  # noqa: E402
from concourse import bacc  # noqa: E402
from concourse.bass_utils import run_bass_kernel_spmd  # noqa: E402
from concourse.masks import make_identity  # noqa: E402
from contextlib import ExitStack  # noqa: E402

B, T, D = 2, 2048, 2048
G, REP, HD = 8, 4, 64
DQ = REP * HD  # 256 q-dims per core
NCORES = 8
P = 128
TB = 512  # q/t block size
KO = D // P  # 16 contraction subtiles for projections
NT = T // TB  # 4 t-blocks
NKT = T // P  # 16 kpos tiles
F32 = mybir.dt.float32
F32R = mybir.dt.float32r
BF16 = mybir.dt.bfloat16
AF = mybir.ActivationFunctionType
SCALE = 1.0 / 8.0  # 1/sqrt(HD)


def r32(ap):
    return ap.bitcast(F32R)


def build_kernel(ctx, tc):
    nc = tc.nc
    xT = nc.dram_tensor("xT", [B, D, T], F32, kind="ExternalInput").ap()
    wq = nc.dram_tensor("wq", [D, DQ], F32, kind="ExternalInput").ap()
    wkv = nc.dram_tensor("wkv", [D, 2 * HD], F32, kind="ExternalInput").ap()
    wo = nc.dram_tensor("wo", [DQ, D], F32, kind="ExternalInput").ap()
    out = nc.dram_tensor("out", [B, T, D], F32, kind="ExternalOutput").ap()

    wpool = ctx.enter_context(tc.tile_pool(name="w", bufs=1))
    qt_pool = ctx.enter_context(tc.tile_pool(name="qt", bufs=2))
    kkt_pool = ctx.enter_context(tc.tile_pool(name="kkt", bufs=2))
    vt_pool = ctx.enter_context(tc.tile_pool(name="vt", bufs=2))
    v_pool = ctx.enter_context(tc.tile_pool(name="v", bufs=2))
    xt_pool = ctx.enter_context(tc.tile_pool(name="xt", bufs=6))
    p_pool = ctx.enter_context(tc.tile_pool(name="p", bufs=3))
    o_pool = ctx.enter_context(tc.tile_pool(name="ot", bufs=2))
    r_pool = ctx.enter_context(tc.tile_pool(name="rcp", bufs=3))
    pp = ctx.enter_context(tc.tile_pool(name="pp", bufs=2, space="PSUM"))
    sp = pp
    op = pp
    wp = pp

    # persistent weights
    wq_sb = wpool.tile([P, KO, DQ], F32R, tag="wq")
    nc.gpsimd.dma_start(wq_sb[:], wq.rearrange("(ko p) m -> p ko m", p=P))
    wkv_sb = wpool.tile([P, KO, 2 * HD], F32R, tag="wkv")
    nc.gpsimd.dma_start(wkv_sb[:], wkv.rearrange("(ko p) m -> p ko m", p=P))
    wo_sb = wpool.tile([P, DQ // P, D], F32R, tag="wo")
    nc.gpsimd.dma_start(wo_sb[:], wo.rearrange("(ko p) m -> p ko m", p=P))
    ident = wpool.tile([P, P], F32, tag="ident")
    make_identity(nc, ident[:])

    for b in range(B):
        # ---------------- projections ----------------
        qt_sb = qt_pool.tile([P, 2, T], F32R, tag="qt")  # QT, scaled by 1/8
        kkt_sb = kkt_pool.tile([P, T], F32R, tag="kkt")  # KT duplicated twice
        vt_sb = vt_pool.tile([P, T], F32, tag="vt")  # VT on partitions 64..127
        for tb in range(NT):
            q_ps0 = pp.tile([P, TB], F32, tag="A")
            q_ps1 = pp.tile([P, TB], F32, tag="B")
            kv_ps = pp.tile([P, TB], F32, tag="C")
            for ko in range(KO):
                xt = xt_pool.tile([P, TB], F32R, tag="xt")
                nc.gpsimd.dma_start(
                    xt[:], xT[b, ko * P : (ko + 1) * P, tb * TB : (tb + 1) * TB]
                )
                st, sp_ = (ko == 0), (ko == KO - 1)
                nc.tensor.matmul(
                    q_ps0[:], wq_sb[:, ko, 0:P], xt[:], start=st, stop=sp_
                )
                nc.tensor.matmul(
                    q_ps1[:], wq_sb[:, ko, P:DQ], xt[:], start=st, stop=sp_
                )
                nc.tensor.matmul(
                    kv_ps[:], wkv_sb[:, ko, :], xt[:], start=st, stop=sp_
                )
            ts = slice(tb * TB, (tb + 1) * TB)
            nc.scalar.activation(qt_sb[:, 0, ts], q_ps0[:], AF.Copy, scale=SCALE)
            nc.scalar.activation(qt_sb[:, 1, ts], q_ps1[:], AF.Copy, scale=SCALE)
            nc.vector.tensor_copy(kkt_sb[0:HD, ts], kv_ps[0:HD, :])
            nc.vector.tensor_copy(vt_sb[HD:P, ts], kv_ps[HD:P, :])
            # duplicate KT to partitions 64..127 (SBUF->SBUF DMA moves partitions)
            nc.sync.dma_start(kkt_sb[HD:P, ts], kkt_sb[0:HD, ts])

        # ---------------- V transpose -> [kpos, hd|1] bf16 ----------------
        v1_sb = v_pool.tile([P, NKT, HD + 1], BF16, tag="v1")
        nc.gpsimd.memset(v1_sb[:, :, HD : HD + 1], 1.0)
        for kt in range(NKT):
            tr_ps = wp.tile([P, TB], F32, tag="D")
            nc.tensor.transpose(
                tr_ps[:, 0:HD],
                vt_sb[HD:P, kt * P : (kt + 1) * P],
                ident[HD:P, HD:P],
            )
            nc.vector.tensor_copy(v1_sb[:, kt, 0:HD], tr_ps[:, 0:HD])

        # ---------------- attention + output proj, per q-block ----------------
        for qb in range(NT):
            qs = slice(qb * TB, (qb + 1) * TB)
            nkt = 4 * (qb + 1)  # causal: kpos tiles 0..nkt-1
            ot_sb = o_pool.tile([P, 2, TB], F32R, tag="ot")
            for pair in range(2):
                o_ps = []
                for i in range(2):
                    o_ps_i = op.tile([P, TB], F32, tag="C", name=f"o_ps_{i}")
                    o_ps.append(o_ps_i)
                for kt in range(nkt):
                    ks = slice(kt * P, (kt + 1) * P)
                    s_ps0 = sp.tile([P, TB], F32, tag="A")
                    s_ps1 = sp.tile([P, TB], F32, tag="B")
                    nc.tensor.matmul(
                        s_ps0[:],
                        kkt_sb[0:HD, ks],
                        qt_sb[0:HD, pair, qs],
                        start=True,
                        stop=True,
                        tile_position=(0, 0),
                    )
                    nc.tensor.matmul(
                        s_ps1[:],
                        kkt_sb[HD:P, ks],
                        qt_sb[HD:P, pair, qs],
                        start=True,
                        stop=True,
                        tile_position=(64, 0),
                    )
                    pt0 = p_pool.tile([P, TB], BF16, tag="p0")
                    pt1 = p_pool.tile([P, TB], BF16, tag="p1")
                    nc.scalar.activation(pt0[:], s_ps0[:], AF.Exp)
                    nc.scalar.activation(pt1[:], s_ps1[:], AF.Exp)
                    if kt >= qb * 4:  # diagonal block: causal mask
                        for pt in (pt0, pt1):
                            nc.gpsimd.affine_select(
                                out=pt[:],
                                in_=pt[:],
                                compare_op=mybir.AluOpType.is_ge,
                                fill=0.0,
                                base=qb * TB - kt * P,
                                channel_multiplier=-1,
                                pattern=[[1, TB]],
                            )
                    st, sp_ = (kt == 0), (kt == nkt - 1)
                    nc.tensor.matmul(
                        o_ps[0][0 : HD + 1, :], v1_sb[:, kt, :], pt0[:],
                        start=st, stop=sp_,
                    )
                    nc.tensor.matmul(
                        o_ps[1][0 : HD + 1, :], v1_sb[:, kt, :], pt1[:],
                        start=st, stop=sp_,
                    )
                # normalize: ot[r] = o_ps[r][:64] / o_ps[r][64]
                for i in range(2):
                    sums = r_pool.tile([1, TB], F32, tag="sums")
                    nc.vector.tensor_copy(sums[:], o_ps[i][HD : HD + 1, :])
                    rb = r_pool.tile([HD, TB], F32, tag="rb")
                    nc.gpsimd.partition_broadcast(rb[:], sums[:])
                    nc.vector.reciprocal(rb[:], rb[:])
                    nc.vector.tensor_mul(
                        ot_sb[i * HD : (i + 1) * HD, pair, :],
                        o_ps[i][0:HD, :],
                        rb[:],
                    )
            # Wo partial for this q-block's 512 tokens
            for tt in range(4):
                rows = slice(qb * TB + tt * P, qb * TB + (tt + 1) * P)
                lslice = slice(tt * P, (tt + 1) * P)
                for nb in range(4):
                    wo_ps = wp.tile([P, TB], F32, tag="D")
                    for ko in range(2):
                        nc.tensor.matmul(
                            wo_ps[:],
                            ot_sb[:, ko, lslice],
                            wo_sb[:, ko, nb * TB : (nb + 1) * TB],
                            start=(ko == 0),
                            stop=(ko == 1),
                        )
                    stg = p_pool.tile([P, TB], F32, tag="stg")
                    nc.vector.tensor_copy(stg[:], wo_ps[:])
                    nc.sync.dma_start(out[b, rows, nb * TB : (nb + 1) * TB], stg[:])


_NC_CACHE = {}


def get_nc():
    if "nc" not in _NC_CACHE:
        nc = bacc.Bacc("TRN2", target_bir_lowering=False, debug=False)
        with tile.TileContext(nc) as tc, ExitStack() as ctx:
            build_kernel(ctx, tc)
        nc.compile()
        _NC_CACHE["nc"] = nc
    return _NC_CACHE["nc"]


def make_in_maps(x, Wq, Wk, Wv, Wo):
    xT = np.ascontiguousarray(np.transpose(np.asarray(x, np.float32), (0, 2, 1)))
    Wq, Wk, Wv, Wo = (np.asarray(w, np.float32) for w in (Wq, Wk, Wv, Wo))
    in_maps = []
    for g in range(NCORES):
        in_maps.append(
            {
                "xT": xT,
                "wq": np.ascontiguousarray(Wq[:, g * DQ : (g + 1) * DQ]),
                "wkv": np.ascontiguousarray(
                    np.concatenate(
                        [Wk[:, g * HD : (g + 1) * HD], Wv[:, g * HD : (g + 1) * HD]],
                        axis=1,
                    )
                ),
                "wo": np.ascontiguousarray(Wo[g * DQ : (g + 1) * DQ, :]),
            }
        )
    return in_maps


def run(x, Wq, Wk, Wv, Wo, trace=False):
    nc = get_nc()
    in_maps = make_in_maps(x, Wq, Wk, Wv, Wo)
    res = run_bass_kernel_spmd(nc, in_maps, list(range(NCORES)), trace=trace)
    acc = np.zeros((B, T, D), np.float32)
    for r in res.results:
        acc += r["out"]
    return acc, res


def kernel(x, Wq, Wk, Wv, Wo):
    return run(x, Wq, Wk, Wv, Wo)[0]
